# revision 16
# baseline (speedup 1.0000x reference)
"""Trainium2 Bass kernel for nn_Encoder_60112362275055 (GRU with skip connections).

B=64, T=512, X=256, H=1024, skip_size=5. Output = 2 * h_{T-1}  -> [64, 1024].

v2 strategy (data-parallel over batch, 8 cores x BL=8, zero cross-core comm):
  - DAG levels with width capped at 8 (list scheduling by height) -> M = w*8 <= 64.
  - Recurrence matmuls in fp16, 2-way column-tiled: the A-half tile (array cols
    0-63) streams W_hh columns for gates [r0,z0,n0] (H-halves 0:512) into psum
    rows 0:M while the B-half tile (cols 64-127) concurrently streams
    [r1,z1,n1] into rows 64:64+M -> 2x effective W_hh stream bandwidth.
  - "2-high" tail: all gate math on [0:64+M, 512] tiles covering both H-halves
    in one op (full partition utilization).
  - b_hh_n folded in via a K=1 ones-row matmul into the n bank.
  - Phase 1 (xi = x @ W_ih.T + bias, fp16) interleaved into the level loop to
    fill PE bubbles.
  - Blends read the previous level's h_new transpose PSUM directly; history
    (fp16) is updated off the critical path.
"""

import sys

import numpy as np

sys.path.insert(0, "/opt/trn_rl_repo")

import concourse.bacc as bacc
import concourse.bass as bass
import concourse.mybir as mybir
from concourse import tile
from concourse.bass_utils import run_bass_kernel_spmd

B, T, X, H = 64, 512, 256, 1024
SKIP = 5
NCORES = 8
BL = B // NCORES  # 8
G3 = 3 * H  # 3072
KC = H // 128  # 8 K-chunks
TBL = T * BL  # 4096
MT = TBL // 128  # 32 phase-1 M-tiles
WCAP = 8
S_HIST = 96

f32 = mybir.dt.float32
f32r = mybir.dt.float32r
fp16 = mybir.dt.float16
AF = mybir.ActivationFunctionType
ALU = mybir.AluOpType


def _skip_plan(T, skip_size):
    slots = np.zeros(T, np.int32)
    use_zero = np.zeros(T, np.float32)
    for i in range(T):
        if i < skip_size:
            if 2 * i < skip_size:
                use_zero[i] = 1.0
            else:
                slots[i] = (skip_size - i) - 1
        else:
            if i - skip_size < skip_size:
                use_zero[i] = 1.0
            else:
                slots[i] = 2 * skip_size - 1
    return slots, use_zero


def _plan(w1, w2):
    """Capped-width topological levels + per-node dependency spec."""
    slots, use_zero = _skip_plan(T, SKIP)
    d1 = np.full(T, -1, np.int64)
    d2 = np.full(T, -1, np.int64)
    for t in range(T):
        if w1[t] == 1 and t - 1 >= 0:
            d1[t] = t - 1
        if w2[t] == 1 and use_zero[t] == 0.0 and t - 1 - slots[t] >= 0:
            d2[t] = t - 1 - slots[t]
    children = [[] for _ in range(T)]
    indeg = np.zeros(T, np.int64)
    for t in range(T):
        for d in (d1[t], d2[t]):
            if d >= 0:
                children[d].append(t)
                indeg[t] += 1
    height = np.zeros(T, np.int64)
    for t in range(T - 1, -1, -1):
        height[t] = 1 + max((height[c] for c in children[t]), default=-1)
    lev_of = np.full(T, -1, np.int64)
    rem = indeg.copy()
    ready = [t for t in range(T) if rem[t] == 0]
    levels = []
    while ready:
        ready.sort(key=lambda t: (-height[t], t))
        take, ready = ready[:WCAP], ready[WCAP:]
        for t in take:
            lev_of[t] = len(levels)
        levels.append(take)
        newly = []
        for t in take:
            for c in children[t]:
                rem[c] -= 1
                if rem[c] == 0:
                    newly.append(c)
        ready.extend(newly)
    assert lev_of.min() >= 0
    order = [t for lv in levels for t in lv]
    slot_of = {t: s for s, t in enumerate(order)}
    pos_of = {}
    for lv in levels:
        for i, t in enumerate(lv):
            pos_of[t] = i

    # circular-history safety
    lev_of_slot = [lev_of[order[s]] for s in range(T)]
    last_use = {}
    for t in range(T):
        for d in (d1[t], d2[t]):
            if d >= 0:
                s = slot_of[d]
                last_use[s] = max(last_use.get(s, -1), int(lev_of[t]))
    for s in range(T):
        s2 = s + S_HIST
        if s2 < T and s in last_use:
            assert lev_of_slot[s2] > last_use[s], (s, s2, "S_HIST too small")
    return levels, order, slot_of, pos_of, lev_of, d1, d2


def _build(levels, slot_of, pos_of, lev_of, d1, d2):
    nc = bacc.Bacc(None)

    xs_d = nc.dram_tensor("xs", [2, 128, TBL], fp16, kind="ExternalInput")
    wih_d = nc.dram_tensor("wih", [2, 128, G3], fp16, kind="ExternalInput")
    whh_d = nc.dram_tensor("whh", [KC, 128, G3], fp16, kind="ExternalInput")
    biasg_d = nc.dram_tensor("biasg", [128, G3], fp16, kind="ExternalInput")
    b2row_d = nc.dram_tensor("b2row", [1, H], fp16, kind="ExternalInput")
    ones_d = nc.dram_tensor("ones", [1, 64], fp16, kind="ExternalInput")
    id32_d = nc.dram_tensor("id32", [128, 128], f32, kind="ExternalInput")
    id16_d = nc.dram_tensor("id16", [128, 128], fp16, kind="ExternalInput")
    out_d = nc.dram_tensor("out", [BL, H], f32, kind="ExternalOutput")
    xi_d = nc.dram_tensor("xi_scratch", [TBL, G3], f32)

    NLEV = len(levels)
    t_out_lev = int(lev_of[T - 1])

    with tile.TileContext(nc) as tc:
        with (
            tc.tile_pool(name="consts", bufs=1) as cpool,
            tc.tile_pool(name="ps", bufs=1, space="PSUM") as pspool,
            tc.tile_pool(name="ptbb", bufs=1, space="PSUM") as bbpool,
            tc.tile_pool(name="ptnw", bufs=2, space="PSUM") as nwpool,
            tc.tile_pool(name="p1ps", bufs=2, space="PSUM") as p1pool,
            tc.tile_pool(name="xiin", bufs=3) as xipool,
            tc.tile_pool(name="p1sb", bufs=2) as p1sb,
            tc.tile_pool(name="ltile", bufs=1) as lpool,
            tc.tile_pool(name="hnewp", bufs=2) as hpool,
            tc.tile_pool(name="hblp", bufs=2) as hblpool,
            tc.tile_pool(name="outp", bufs=1) as opool,
        ):
            whh = cpool.tile([128, KC * G3], fp16)
            nc.sync.dma_start(
                whh[:].rearrange("p (k f) -> p k f", k=KC),
                whh_d.rearrange("k p f -> p k f"),
            )
            xs = cpool.tile([128, 2 * TBL], fp16)
            nc.sync.dma_start(
                xs[:].rearrange("p (k f) -> p k f", k=2),
                xs_d.rearrange("k p f -> p k f"),
            )
            wih = cpool.tile([128, 2 * G3], fp16)
            nc.sync.dma_start(
                wih[:].rearrange("p (k f) -> p k f", k=2),
                wih_d.rearrange("k p f -> p k f"),
            )
            biasg = cpool.tile([128, G3], fp16)
            nc.sync.dma_start(biasg[:], biasg_d[:])
            b2row = cpool.tile([1, H], fp16)
            nc.sync.dma_start(b2row[:], b2row_d[:])
            ones = cpool.tile([1, 64], fp16)
            nc.sync.dma_start(ones[:], ones_d[:])
            id32 = cpool.tile([128, 128], f32)
            nc.sync.dma_start(id32[:], id32_d[:])
            id16 = cpool.tile([128, 128], fp16)
            nc.sync.dma_start(id16[:], id16_d[:])
            hist = cpool.tile([128, S_HIST * 64], fp16)

            # ---- phase-1 job machinery: xi chunk (m, nb) ----
            p1_jobs = [(m, nb) for m in range(MT) for nb in range(6)]
            p1_state = {"next": 0}

            def emit_p1(n):
                j0 = p1_state["next"]
                for jj in range(j0, min(j0 + n, len(p1_jobs))):
                    m, nb = p1_jobs[jj]
                    ps1 = p1pool.tile([128, 512], f32, tag="p1ps", name=f"p1ps_{jj}")
                    nc.tensor.matmul(
                        ps1[:],
                        xs[:, m * 128 : (m + 1) * 128],
                        wih[:, nb * 512 : (nb + 1) * 512],
                        start=True,
                        stop=False,
                    )
                    nc.tensor.matmul(
                        ps1[:],
                        xs[:, TBL + m * 128 : TBL + (m + 1) * 128],
                        wih[:, G3 + nb * 512 : G3 + (nb + 1) * 512],
                        start=False,
                        stop=True,
                    )
                    sb1 = p1sb.tile([128, 512], f32, tag="p1sb", name=f"p1sb_{jj}")
                    nc.vector.tensor_add(
                        sb1[:], ps1[:], biasg[:, nb * 512 : (nb + 1) * 512]
                    )
                    nc.sync.dma_start(
                        xi_d[m * 128 : (m + 1) * 128, nb * 512 : (nb + 1) * 512],
                        sb1[:],
                    )
                p1_state["next"] = min(j0 + n, len(p1_jobs))

            def p1_tiles_done():
                return p1_state["next"] // 6  # fully-emitted m-tiles (floor)

            emit_p1(6 * 5)  # pre-run first 5 m-tiles

            # hist slot layout: col(slot, q, h, b) = slot*64 + q*16 + h*8 + b
            def hist_node(s):
                col = (s % S_HIST) * 64
                return hist[:, col : col + 64].rearrange(
                    "p (q h b) -> p q h b", q=4, h=2
                )

            # hbl layout: [128, q(4) * 128]; chunk c at q*128 + (c//4)*64
            def hbl_node(hbl_ap, i):
                return hbl_ap.rearrange("p (q h x) -> p q h x", q=4, h=2)[
                    :, :, :, i * BL : (i + 1) * BL
                ]

            pt_prev = [None]

            def pt_node(pt_ap, i):
                return pt_ap.rearrange("p (q h x) -> p q h x", q=4, h=2)[
                    :, :, :, i * BL : (i + 1) * BL
                ]

            xi_tiles = {}

            def load_xi(l):
                nodes = levels[l]
                w = len(nodes)
                M = w * BL
                s0 = slot_of[nodes[0]]
                need_tile = ((s0 + w) * BL + 127) // 128
                if p1_tiles_done() < need_tile:
                    emit_p1(6 * (need_tile - p1_tiles_done()))
                xt = xipool.tile([128, 1536], f32, tag="xi", name=f"xi_{l}")
                nc.sync.dma_start(
                    xt[0:M, :], xi_d[s0 * BL : s0 * BL + M, 0:1536]
                )
                nc.sync.dma_start(
                    xt[64 : 64 + M, :], xi_d[s0 * BL : s0 * BL + M, 1536:3072]
                )
                xi_tiles[l] = xt

            load_xi(0)
            load_xi(1)

            for l, nodes in enumerate(levels):
                w = len(nodes)
                M = w * BL
                P2 = 64 + M
                s0 = slot_of[nodes[0]]

                # ---------- blend: hbl = sum of deps (fp16, 4D strided) ----------
                hbl = hblpool.tile([128, 4 * 128], fp16, tag="hbl", name=f"hbl_{l}")
                for i, t in enumerate(nodes):
                    dst = hbl_node(hbl[:], i)
                    use_dve = (i % 2 == 0) and pt_prev[0] is not None
                    srcs = []
                    for j, d in enumerate((int(d1[t]), int(d2[t]))):
                        if d < 0:
                            continue
                        if use_dve and j == 0 and lev_of[d] == l - 1:
                            srcs.append(pt_node(pt_prev[0][:], pos_of[d]))
                        else:
                            srcs.append(hist_node(slot_of[d]))
                    eng = nc.vector if use_dve else nc.gpsimd
                    if len(srcs) == 0:
                        nc.gpsimd.memset(dst, 0.0)
                    elif len(srcs) == 1:
                        eng.tensor_copy(dst, srcs[0])
                    else:
                        eng.tensor_add(dst, srcs[0], srcs[1])

                # ---------- hbb transposes (hbl -> batch-major 2-high psum) ----------
                pt_bb = bbpool.tile([128, 512], fp16, tag="ptbb", name=f"ptbb_{l}")
                for q in range(4):
                    nc.tensor.transpose(
                        pt_bb[0:P2, q * 128 : (q + 1) * 128],
                        hbl[:, q * 128 : q * 128 + P2],
                        id16[:128, :128],
                    )

                # ---------- recurrence matmuls: fp16, 2-way col-tiled ----------
                ps_rz = pspool.tile([128, 1024], f32, tag="psrz", name=f"psrz_{l}")
                ps_n = pspool.tile([128, 512], f32, tag="psn", name=f"psn_{l}")

                def hbl_chunk(c):
                    off = (c // 4) * 64
                    q = c % 4
                    return hbl[:, q * 128 + off : q * 128 + off + M]

                for b in range(3):  # r, z, n banks
                    dstA = (
                        ps_rz[0:M, b * 512 : (b + 1) * 512]
                        if b < 2
                        else ps_n[0:M, :]
                    )
                    dstB = (
                        ps_rz[64 : 64 + M, b * 512 : (b + 1) * 512]
                        if b < 2
                        else ps_n[64 : 64 + M, :]
                    )
                    for k in range(KC):
                        nc.tensor.matmul(
                            dstA,
                            hbl_chunk(k),
                            whh[:, k * G3 + b * 512 : k * G3 + (b + 1) * 512],
                            start=(k == 0),
                            stop=(k == KC - 1 and b != 2),
                            tile_position=(0, 0),
                        )
                        nc.tensor.matmul(
                            dstB,
                            hbl_chunk(k),
                            whh[
                                :,
                                k * G3 + 1536 + b * 512 : k * G3 + 1536 + (b + 1) * 512,
                            ],
                            start=(k == 0),
                            stop=(k == KC - 1 and b != 2),
                            tile_position=(0, 64),
                        )
                    if b == 2:  # + b_hh_n via K=1 ones-row matmul
                        nc.tensor.matmul(
                            dstA,
                            ones[0:1, 0:M],
                            b2row[0:1, 0:512],
                            start=False,
                            stop=True,
                            tile_position=(0, 0),
                        )
                        nc.tensor.matmul(
                            dstB,
                            ones[0:1, 0:M],
                            b2row[0:1, 512:1024],
                            start=False,
                            stop=True,
                            tile_position=(0, 64),
                        )

                xt = xi_tiles.pop(l)
                if l + 2 < NLEV:
                    load_xi(l + 2)

                # ---------- tail (2-high, fp32) ----------
                rz = lpool.tile([128, 1024], f32, tag="rz", name=f"rz_{l}")
                nc.vector.tensor_add(
                    rz[0:P2, 0:512], ps_rz[0:P2, 0:512], xt[0:P2, 0:512]
                )
                nc.scalar.activation(
                    rz[0:P2, 0:512], rz[0:P2, 0:512], AF.Sigmoid
                )
                nc.vector.tensor_add(
                    rz[0:P2, 512:1024], ps_rz[0:P2, 512:1024], xt[0:P2, 512:1024]
                )
                nc.scalar.activation(
                    rz[0:P2, 512:1024], rz[0:P2, 512:1024], AF.Sigmoid
                )
                w2t = lpool.tile([128, 512], fp16, tag="w2t", name=f"w2t_{l}")
                nc.scalar.activation(
                    w2t[0:P2, :], rz[0:P2, 512:1024], AF.Copy, bias=1.0, scale=-1.0
                )
                qt = lpool.tile([128, 512], f32, tag="qt", name=f"qt_{l}")
                nc.vector.tensor_mul(qt[0:P2, :], rz[0:P2, 512:1024], pt_bb[0:P2, :])

                hnew = hpool.tile([128, 512], f32, tag="hnew", name=f"hnew_{l}")
                pt_nw = nwpool.tile([128, 4 * 128], f32, tag="ptnw", name=f"ptnw_{l}")
                nt = lpool.tile([128, 512], f32, tag="nt", name=f"nt_{l}")
                nc.vector.tensor_mul(nt[0:P2, :], ps_n[0:P2, :], rz[0:P2, 0:512])
                nc.vector.tensor_add(nt[0:P2, :], nt[0:P2, :], xt[0:P2, 1024:1536])
                nc.scalar.activation(nt[0:P2, :], nt[0:P2, :], AF.Tanh)
                nc.vector.tensor_mul(nt[0:P2, :], nt[0:P2, :], w2t[0:P2, :])
                nc.vector.tensor_add(hnew[0:P2, :], nt[0:P2, :], qt[0:P2, :])
                for qq in range(4):
                    nc.tensor.transpose(
                        pt_nw[:, qq * 128 : qq * 128 + P2],
                        hnew[0:P2, qq * 128 : (qq + 1) * 128],
                        id32[0:P2, 0:P2],
                    )
                sm = s0 % S_HIST
                n1 = min(w, S_HIST - sm)
                hist5 = hist[:].rearrange(
                    "p (s q h b) -> p s q h b", s=S_HIST, q=4, h=2
                )
                for qq in range(4):
                    src_pt = pt_nw[:, qq * 128 : qq * 128 + 128].rearrange(
                        "p (h x) -> p h x", h=2
                    )
                    dst = hist5[:, sm : sm + n1, qq, :, :].rearrange(
                        "p s h b -> p h s b"
                    )
                    nc.scalar.copy(
                        dst,
                        src_pt[:, :, 0 : n1 * BL].rearrange(
                            "p h (s b) -> p h s b", b=BL
                        ),
                    )
                    if n1 < w:
                        n2 = w - n1
                        dst2 = hist5[:, 0:n2, qq, :, :].rearrange(
                            "p s h b -> p h s b"
                        )
                        nc.scalar.copy(
                            dst2,
                            src_pt[:, :, n1 * BL : w * BL].rearrange(
                                "p h (s b) -> p h s b", b=BL
                            ),
                        )
                pt_prev[0] = pt_nw

                if l == t_out_lev:
                    i = nodes.index(T - 1)
                    outt = opool.tile([128, 512], f32, tag="outt")
                    nc.scalar.activation(
                        outt[0:P2, :], hnew[0:P2, :], AF.Copy, scale=2.0
                    )
                    nc.sync.dma_start(
                        out_d[:, 0:512], outt[i * BL : (i + 1) * BL, :]
                    )
                    nc.sync.dma_start(
                        out_d[:, 512:1024], outt[64 + i * BL : 64 + (i + 1) * BL, :]
                    )

            emit_p1(len(p1_jobs))  # flush any remaining phase-1 work

    nc.finalize()
    return nc


def kernel(**inputs):
    x = np.asarray(inputs["x"], np.float32)
    W_ih = np.asarray(inputs["W_ih"], np.float32)
    W_hh = np.asarray(inputs["W_hh"], np.float32)
    b_ih = np.asarray(inputs["b_ih"], np.float32)
    b_hh = np.asarray(inputs["b_hh"], np.float32)
    w1 = np.asarray(inputs["w1"], np.int32)
    w2 = np.asarray(inputs["w2"], np.int32)

    levels, order, slot_of, pos_of, lev_of, d1, d2 = _plan(w1, w2)
    assert max(len(lv) for lv in levels) * BL <= 64
    nc = _build(levels, slot_of, pos_of, lev_of, d1, d2)

    # gate permutation: [r0, z0, n0, r1, z1, n1] (512-wide pieces)
    perm = np.concatenate(
        [
            np.arange(0, 512),
            np.arange(1024, 1536),
            np.arange(2048, 2560),
            np.arange(512, 1024),
            np.arange(1536, 2048),
            np.arange(2560, 3072),
        ]
    )
    bias = (b_ih + b_hh).copy()
    bias[2 * H :] = b_ih[2 * H :]  # n-part: only b_ih (b_hh_n applied inside)
    whh_t = np.ascontiguousarray(W_hh[perm].T.reshape(KC, 128, G3)).astype(np.float16)
    wih_t = np.ascontiguousarray(W_ih[perm].T.reshape(2, 128, G3)).astype(np.float16)
    biasg = np.broadcast_to(bias[perm], (128, G3)).astype(np.float16).copy()
    b2row = b_hh[2 * H :].astype(np.float16).reshape(1, H)
    ones = np.ones((1, 64), np.float16)
    id32 = np.eye(128, dtype=np.float32)
    id16 = np.eye(128, dtype=np.float16)

    in_maps = []
    for c in range(NCORES):
        xc = x[c * BL : (c + 1) * BL]  # [8, T, X]
        xsrt = xc[:, order, :]  # level-sorted
        xs = np.ascontiguousarray(
            xsrt.transpose(2, 1, 0).reshape(2, 128, TBL)
        ).astype(np.float16)
        in_maps.append(
            {
                "xs": xs,
                "wih": wih_t,
                "whh": whh_t,
                "biasg": biasg,
                "b2row": b2row,
                "ones": ones,
                "id32": id32,
                "id16": id16,
            }
        )
    res = run_bass_kernel_spmd(nc, in_maps, core_ids=list(range(NCORES)))
    if getattr(res, "exec_time_ns", None):
        print("HW exec time:", res.exec_time_ns, "ns")
    global LAST_RESULT
    LAST_RESULT = res
    out = np.concatenate([res.results[c]["out"] for c in range(NCORES)], axis=0)
    return out.astype(np.float32)


LAST_RESULT = None


if __name__ == "__main__":
    rng = np.random.default_rng(0)
    ins = {
        "x": rng.standard_normal((B, T, X)).astype(np.float32),
        "W_ih": rng.standard_normal((G3, X)).astype(np.float32) / 32,
        "W_hh": rng.standard_normal((G3, H)).astype(np.float32) / 32,
        "b_ih": rng.standard_normal(G3).astype(np.float32) / 32,
        "b_hh": rng.standard_normal(G3).astype(np.float32) / 32,
        "w1": rng.integers(0, 2, T).astype(np.int32),
        "w2": rng.integers(0, 2, T).astype(np.int32),
        "skip_size": 5,
    }
    ins["w2"] = np.where(ins["w1"] == 0, 1, ins["w2"]).astype(np.int32)
    out = kernel(**ins)
    print("ran", out.shape, out.dtype, float(np.abs(out).mean()))


# revision 17
# speedup vs baseline: 1.0174x; 1.0174x over previous
"""Trainium2 Bass kernel for nn_Encoder_60112362275055 (GRU with skip connections).

B=64, T=512, X=256, H=1024, skip_size=5. Output = 2 * h_{T-1}  -> [64, 1024].

v2 strategy (data-parallel over batch, 8 cores x BL=8, zero cross-core comm):
  - DAG levels with width capped at 8 (list scheduling by height) -> M = w*8 <= 64.
  - Recurrence matmuls in fp16, 2-way column-tiled: the A-half tile (array cols
    0-63) streams W_hh columns for gates [r0,z0,n0] (H-halves 0:512) into psum
    rows 0:M while the B-half tile (cols 64-127) concurrently streams
    [r1,z1,n1] into rows 64:64+M -> 2x effective W_hh stream bandwidth.
  - "2-high" tail: all gate math on [0:64+M, 512] tiles covering both H-halves
    in one op (full partition utilization).
  - b_hh_n folded in via a K=1 ones-row matmul into the n bank.
  - Phase 1 (xi = x @ W_ih.T + bias, fp16) interleaved into the level loop to
    fill PE bubbles.
  - Blends read the previous level's h_new transpose PSUM directly; history
    (fp16) is updated off the critical path.
"""

import sys

import numpy as np

sys.path.insert(0, "/opt/trn_rl_repo")

import concourse.bacc as bacc
import concourse.bass as bass
import concourse.mybir as mybir
from concourse import tile
from concourse.bass_utils import run_bass_kernel_spmd

B, T, X, H = 64, 512, 256, 1024
SKIP = 5
NCORES = 8
BL = B // NCORES  # 8
G3 = 3 * H  # 3072
KC = H // 128  # 8 K-chunks
TBL = T * BL  # 4096
MT = TBL // 128  # 32 phase-1 M-tiles
WCAP = 8
S_HIST = 96

f32 = mybir.dt.float32
f32r = mybir.dt.float32r
fp16 = mybir.dt.float16
AF = mybir.ActivationFunctionType
ALU = mybir.AluOpType


def _skip_plan(T, skip_size):
    slots = np.zeros(T, np.int32)
    use_zero = np.zeros(T, np.float32)
    for i in range(T):
        if i < skip_size:
            if 2 * i < skip_size:
                use_zero[i] = 1.0
            else:
                slots[i] = (skip_size - i) - 1
        else:
            if i - skip_size < skip_size:
                use_zero[i] = 1.0
            else:
                slots[i] = 2 * skip_size - 1
    return slots, use_zero


def _plan(w1, w2):
    """Capped-width topological levels + per-node dependency spec."""
    slots, use_zero = _skip_plan(T, SKIP)
    d1 = np.full(T, -1, np.int64)
    d2 = np.full(T, -1, np.int64)
    for t in range(T):
        if w1[t] == 1 and t - 1 >= 0:
            d1[t] = t - 1
        if w2[t] == 1 and use_zero[t] == 0.0 and t - 1 - slots[t] >= 0:
            d2[t] = t - 1 - slots[t]
    children = [[] for _ in range(T)]
    indeg = np.zeros(T, np.int64)
    for t in range(T):
        for d in (d1[t], d2[t]):
            if d >= 0:
                children[d].append(t)
                indeg[t] += 1
    height = np.zeros(T, np.int64)
    for t in range(T - 1, -1, -1):
        height[t] = 1 + max((height[c] for c in children[t]), default=-1)
    lev_of = np.full(T, -1, np.int64)
    rem = indeg.copy()
    ready = [t for t in range(T) if rem[t] == 0]
    levels = []
    while ready:
        ready.sort(key=lambda t: (-height[t], t))
        take, ready = ready[:WCAP], ready[WCAP:]
        for t in take:
            lev_of[t] = len(levels)
        levels.append(take)
        newly = []
        for t in take:
            for c in children[t]:
                rem[c] -= 1
                if rem[c] == 0:
                    newly.append(c)
        ready.extend(newly)
    assert lev_of.min() >= 0
    order = [t for lv in levels for t in lv]
    slot_of = {t: s for s, t in enumerate(order)}
    pos_of = {}
    for lv in levels:
        for i, t in enumerate(lv):
            pos_of[t] = i

    # circular-history safety
    lev_of_slot = [lev_of[order[s]] for s in range(T)]
    last_use = {}
    for t in range(T):
        for d in (d1[t], d2[t]):
            if d >= 0:
                s = slot_of[d]
                last_use[s] = max(last_use.get(s, -1), int(lev_of[t]))
    for s in range(T):
        s2 = s + S_HIST
        if s2 < T and s in last_use:
            assert lev_of_slot[s2] > last_use[s], (s, s2, "S_HIST too small")
    return levels, order, slot_of, pos_of, lev_of, d1, d2


def _build(levels, slot_of, pos_of, lev_of, d1, d2):
    nc = bacc.Bacc(None)

    xs_d = nc.dram_tensor("xs", [2, 128, TBL], fp16, kind="ExternalInput")
    wih_d = nc.dram_tensor("wih", [2, 128, G3], fp16, kind="ExternalInput")
    whh_d = nc.dram_tensor("whh", [KC, 128, G3], fp16, kind="ExternalInput")
    biasg_d = nc.dram_tensor("biasg", [128, G3], fp16, kind="ExternalInput")
    b2row_d = nc.dram_tensor("b2row", [1, H], fp16, kind="ExternalInput")
    ones_d = nc.dram_tensor("ones", [1, 64], fp16, kind="ExternalInput")
    id32_d = nc.dram_tensor("id32", [128, 128], f32, kind="ExternalInput")
    id16_d = nc.dram_tensor("id16", [128, 128], fp16, kind="ExternalInput")
    out_d = nc.dram_tensor("out", [BL, H], f32, kind="ExternalOutput")
    xi_d = nc.dram_tensor("xi_scratch", [TBL, G3], f32)

    NLEV = len(levels)
    t_out_lev = int(lev_of[T - 1])

    with tile.TileContext(nc) as tc:
        with (
            tc.tile_pool(name="consts", bufs=1) as cpool,
            tc.tile_pool(name="ps", bufs=1, space="PSUM") as pspool,
            tc.tile_pool(name="ptbb", bufs=1, space="PSUM") as bbpool,
            tc.tile_pool(name="ptnw", bufs=2, space="PSUM") as nwpool,
            tc.tile_pool(name="p1ps", bufs=2, space="PSUM") as p1pool,
            tc.tile_pool(name="xiin", bufs=3) as xipool,
            tc.tile_pool(name="p1sb", bufs=2) as p1sb,
            tc.tile_pool(name="ltile", bufs=1) as lpool,
            tc.tile_pool(name="hnewp", bufs=2) as hpool,
            tc.tile_pool(name="hblp", bufs=2) as hblpool,
            tc.tile_pool(name="outp", bufs=1) as opool,
        ):
            whh = cpool.tile([128, KC * G3], fp16)
            nc.sync.dma_start(
                whh[:].rearrange("p (k f) -> p k f", k=KC),
                whh_d.rearrange("k p f -> p k f"),
            )
            xs = cpool.tile([128, 2 * TBL], fp16)
            nc.sync.dma_start(
                xs[:].rearrange("p (k f) -> p k f", k=2),
                xs_d.rearrange("k p f -> p k f"),
            )
            wih = cpool.tile([128, 2 * G3], fp16)
            nc.sync.dma_start(
                wih[:].rearrange("p (k f) -> p k f", k=2),
                wih_d.rearrange("k p f -> p k f"),
            )
            biasg = cpool.tile([128, G3], fp16)
            nc.sync.dma_start(biasg[:], biasg_d[:])
            b2row = cpool.tile([1, H], fp16)
            nc.sync.dma_start(b2row[:], b2row_d[:])
            ones = cpool.tile([1, 64], fp16)
            nc.sync.dma_start(ones[:], ones_d[:])
            id32 = cpool.tile([128, 128], f32)
            nc.sync.dma_start(id32[:], id32_d[:])
            id16 = cpool.tile([128, 128], fp16)
            nc.sync.dma_start(id16[:], id16_d[:])
            hist = cpool.tile([128, S_HIST * 64], fp16)

            # ---- phase-1 job machinery: xi chunk (m, nb) ----
            p1_jobs = [(m, nb) for m in range(MT) for nb in range(6)]
            p1_state = {"next": 0}

            def emit_p1(n):
                j0 = p1_state["next"]
                for jj in range(j0, min(j0 + n, len(p1_jobs))):
                    m, nb = p1_jobs[jj]
                    ps1 = p1pool.tile([128, 512], f32, tag="p1ps", name=f"p1ps_{jj}")
                    nc.tensor.matmul(
                        ps1[:],
                        xs[:, m * 128 : (m + 1) * 128],
                        wih[:, nb * 512 : (nb + 1) * 512],
                        start=True,
                        stop=False,
                    )
                    nc.tensor.matmul(
                        ps1[:],
                        xs[:, TBL + m * 128 : TBL + (m + 1) * 128],
                        wih[:, G3 + nb * 512 : G3 + (nb + 1) * 512],
                        start=False,
                        stop=True,
                    )
                    sb1 = p1sb.tile([128, 512], f32, tag="p1sb", name=f"p1sb_{jj}")
                    nc.vector.tensor_add(
                        sb1[:], ps1[:], biasg[:, nb * 512 : (nb + 1) * 512]
                    )
                    nc.sync.dma_start(
                        xi_d[m * 128 : (m + 1) * 128, nb * 512 : (nb + 1) * 512],
                        sb1[:],
                    )
                p1_state["next"] = min(j0 + n, len(p1_jobs))

            def p1_tiles_done():
                return p1_state["next"] // 6  # fully-emitted m-tiles (floor)

            emit_p1(6 * 5)  # pre-run first 5 m-tiles

            # hist slot layout: col(slot, q, h, b) = slot*64 + q*16 + h*8 + b
            def hist_node(s):
                col = (s % S_HIST) * 64
                return hist[:, col : col + 64].rearrange(
                    "p (q h b) -> p q h b", q=4, h=2
                )

            # hbl layout: [128, q(4) * 128]; chunk c at q*128 + (c//4)*64
            def hbl_node(hbl_ap, i):
                return hbl_ap.rearrange("p (q h x) -> p q h x", q=4, h=2)[
                    :, :, :, i * BL : (i + 1) * BL
                ]

            pt_prev = [None]

            def pt_node(pt_ap, i):
                return pt_ap.rearrange("p (q h x) -> p q h x", q=4, h=2)[
                    :, :, :, i * BL : (i + 1) * BL
                ]

            xi_tiles = {}

            def load_xi(l):
                nodes = levels[l]
                w = len(nodes)
                M = w * BL
                s0 = slot_of[nodes[0]]
                need_tile = ((s0 + w) * BL + 127) // 128
                if p1_tiles_done() < need_tile:
                    emit_p1(6 * (need_tile - p1_tiles_done()))
                xt = xipool.tile([128, 1536], f32, tag="xi", name=f"xi_{l}")
                nc.sync.dma_start(
                    xt[0:M, :], xi_d[s0 * BL : s0 * BL + M, 0:1536]
                )
                nc.sync.dma_start(
                    xt[64 : 64 + M, :], xi_d[s0 * BL : s0 * BL + M, 1536:3072]
                )
                xi_tiles[l] = xt

            load_xi(0)
            load_xi(1)

            for l, nodes in enumerate(levels):
                w = len(nodes)
                M = w * BL
                P2 = 64 + M
                s0 = slot_of[nodes[0]]

                # ---------- blend: hbl = sum of deps (fp16, 4D strided) ----------
                hbl = hblpool.tile([128, 4 * 128], fp16, tag="hbl", name=f"hbl_{l}")
                for i, t in enumerate(nodes):
                    dst = hbl_node(hbl[:], i)
                    srcs = []
                    n_pt = 0
                    for d in (int(d1[t]), int(d2[t])):
                        if d < 0:
                            continue
                        if (
                            lev_of[d] == l - 1
                            and pt_prev[0] is not None
                            and n_pt == 0
                        ):
                            srcs.append(pt_node(pt_prev[0][:], pos_of[d]))
                            n_pt += 1
                        else:
                            srcs.append(hist_node(slot_of[d]))
                    eng = nc.vector if n_pt else nc.gpsimd
                    if len(srcs) == 0:
                        nc.gpsimd.memset(dst, 0.0)
                    elif len(srcs) == 1:
                        eng.tensor_copy(dst, srcs[0])
                    else:
                        eng.tensor_add(dst, srcs[0], srcs[1])

                # ---------- hbb transposes (hbl -> batch-major 2-high psum) ----------
                pt_bb = bbpool.tile([128, 512], fp16, tag="ptbb", name=f"ptbb_{l}")
                for q in range(4):
                    nc.tensor.transpose(
                        pt_bb[0:P2, q * 128 : (q + 1) * 128],
                        hbl[:, q * 128 : q * 128 + P2],
                        id16[:128, :128],
                    )

                # ---------- recurrence matmuls: fp16, 2-way col-tiled ----------
                ps_rz = pspool.tile([128, 1024], f32, tag="psrz", name=f"psrz_{l}")
                ps_n = pspool.tile([128, 512], f32, tag="psn", name=f"psn_{l}")

                def hbl_chunk(c):
                    off = (c // 4) * 64
                    q = c % 4
                    return hbl[:, q * 128 + off : q * 128 + off + M]

                for b in range(3):  # r, z, n banks
                    dstA = (
                        ps_rz[0:M, b * 512 : (b + 1) * 512]
                        if b < 2
                        else ps_n[0:M, :]
                    )
                    dstB = (
                        ps_rz[64 : 64 + M, b * 512 : (b + 1) * 512]
                        if b < 2
                        else ps_n[64 : 64 + M, :]
                    )
                    for k in range(KC):
                        nc.tensor.matmul(
                            dstA,
                            hbl_chunk(k),
                            whh[:, k * G3 + b * 512 : k * G3 + (b + 1) * 512],
                            start=(k == 0),
                            stop=(k == KC - 1 and b != 2),
                            tile_position=(0, 0),
                        )
                        nc.tensor.matmul(
                            dstB,
                            hbl_chunk(k),
                            whh[
                                :,
                                k * G3 + 1536 + b * 512 : k * G3 + 1536 + (b + 1) * 512,
                            ],
                            start=(k == 0),
                            stop=(k == KC - 1 and b != 2),
                            tile_position=(0, 64),
                        )
                    if b == 2:  # + b_hh_n via K=1 ones-row matmul
                        nc.tensor.matmul(
                            dstA,
                            ones[0:1, 0:M],
                            b2row[0:1, 0:512],
                            start=False,
                            stop=True,
                            tile_position=(0, 0),
                        )
                        nc.tensor.matmul(
                            dstB,
                            ones[0:1, 0:M],
                            b2row[0:1, 512:1024],
                            start=False,
                            stop=True,
                            tile_position=(0, 64),
                        )

                xt = xi_tiles.pop(l)
                if l + 2 < NLEV:
                    load_xi(l + 2)

                # ---------- tail (2-high, fp32) ----------
                rz = lpool.tile([128, 1024], f32, tag="rz", name=f"rz_{l}")
                nc.vector.tensor_add(
                    rz[0:P2, 0:512], ps_rz[0:P2, 0:512], xt[0:P2, 0:512]
                )
                nc.scalar.activation(
                    rz[0:P2, 0:512], rz[0:P2, 0:512], AF.Sigmoid
                )
                nc.vector.tensor_add(
                    rz[0:P2, 512:1024], ps_rz[0:P2, 512:1024], xt[0:P2, 512:1024]
                )
                nc.scalar.activation(
                    rz[0:P2, 512:1024], rz[0:P2, 512:1024], AF.Sigmoid
                )
                w2t = lpool.tile([128, 512], fp16, tag="w2t", name=f"w2t_{l}")
                nc.scalar.activation(
                    w2t[0:P2, :], rz[0:P2, 512:1024], AF.Copy, bias=1.0, scale=-1.0
                )
                qt = lpool.tile([128, 512], f32, tag="qt", name=f"qt_{l}")
                nc.vector.tensor_mul(qt[0:P2, :], rz[0:P2, 512:1024], pt_bb[0:P2, :])

                hnew = hpool.tile([128, 512], fp16, tag="hnew", name=f"hnew_{l}")
                pt_nw = nwpool.tile([128, 4 * 128], fp16, tag="ptnw", name=f"ptnw_{l}")
                nt = lpool.tile([128, 512], f32, tag="nt", name=f"nt_{l}")
                nc.vector.tensor_mul(nt[0:P2, :], ps_n[0:P2, :], rz[0:P2, 0:512])
                nc.vector.tensor_add(nt[0:P2, :], nt[0:P2, :], xt[0:P2, 1024:1536])
                nc.scalar.activation(nt[0:P2, :], nt[0:P2, :], AF.Tanh)
                nc.vector.tensor_mul(nt[0:P2, :], nt[0:P2, :], w2t[0:P2, :])
                nc.vector.tensor_add(hnew[0:P2, :], nt[0:P2, :], qt[0:P2, :])
                for qq in range(4):
                    nc.tensor.transpose(
                        pt_nw[:, qq * 128 : qq * 128 + P2],
                        hnew[0:P2, qq * 128 : (qq + 1) * 128],
                        id16[0:P2, 0:P2],
                    )
                sm = s0 % S_HIST
                n1 = min(w, S_HIST - sm)
                hist5 = hist[:].rearrange(
                    "p (s q h b) -> p s q h b", s=S_HIST, q=4, h=2
                )
                for qq in range(4):
                    src_pt = pt_nw[:, qq * 128 : qq * 128 + 128].rearrange(
                        "p (h x) -> p h x", h=2
                    )
                    dst = hist5[:, sm : sm + n1, qq, :, :].rearrange(
                        "p s h b -> p h s b"
                    )
                    nc.scalar.copy(
                        dst,
                        src_pt[:, :, 0 : n1 * BL].rearrange(
                            "p h (s b) -> p h s b", b=BL
                        ),
                    )
                    if n1 < w:
                        n2 = w - n1
                        dst2 = hist5[:, 0:n2, qq, :, :].rearrange(
                            "p s h b -> p h s b"
                        )
                        nc.scalar.copy(
                            dst2,
                            src_pt[:, :, n1 * BL : w * BL].rearrange(
                                "p h (s b) -> p h s b", b=BL
                            ),
                        )
                pt_prev[0] = pt_nw

                if l == t_out_lev:
                    i = nodes.index(T - 1)
                    outt = opool.tile([128, 512], f32, tag="outt")
                    nc.scalar.activation(
                        outt[0:P2, :], hnew[0:P2, :], AF.Copy, scale=2.0
                    )
                    nc.sync.dma_start(
                        out_d[:, 0:512], outt[i * BL : (i + 1) * BL, :]
                    )
                    nc.sync.dma_start(
                        out_d[:, 512:1024], outt[64 + i * BL : 64 + (i + 1) * BL, :]
                    )

            emit_p1(len(p1_jobs))  # flush any remaining phase-1 work

    nc.finalize()
    return nc


def kernel(**inputs):
    x = np.asarray(inputs["x"], np.float32)
    W_ih = np.asarray(inputs["W_ih"], np.float32)
    W_hh = np.asarray(inputs["W_hh"], np.float32)
    b_ih = np.asarray(inputs["b_ih"], np.float32)
    b_hh = np.asarray(inputs["b_hh"], np.float32)
    w1 = np.asarray(inputs["w1"], np.int32)
    w2 = np.asarray(inputs["w2"], np.int32)

    levels, order, slot_of, pos_of, lev_of, d1, d2 = _plan(w1, w2)
    assert max(len(lv) for lv in levels) * BL <= 64
    nc = _build(levels, slot_of, pos_of, lev_of, d1, d2)

    # gate permutation: [r0, z0, n0, r1, z1, n1] (512-wide pieces)
    perm = np.concatenate(
        [
            np.arange(0, 512),
            np.arange(1024, 1536),
            np.arange(2048, 2560),
            np.arange(512, 1024),
            np.arange(1536, 2048),
            np.arange(2560, 3072),
        ]
    )
    bias = (b_ih + b_hh).copy()
    bias[2 * H :] = b_ih[2 * H :]  # n-part: only b_ih (b_hh_n applied inside)
    whh_t = np.ascontiguousarray(W_hh[perm].T.reshape(KC, 128, G3)).astype(np.float16)
    wih_t = np.ascontiguousarray(W_ih[perm].T.reshape(2, 128, G3)).astype(np.float16)
    biasg = np.broadcast_to(bias[perm], (128, G3)).astype(np.float16).copy()
    b2row = b_hh[2 * H :].astype(np.float16).reshape(1, H)
    ones = np.ones((1, 64), np.float16)
    id32 = np.eye(128, dtype=np.float32)
    id16 = np.eye(128, dtype=np.float16)

    in_maps = []
    for c in range(NCORES):
        xc = x[c * BL : (c + 1) * BL]  # [8, T, X]
        xsrt = xc[:, order, :]  # level-sorted
        xs = np.ascontiguousarray(
            xsrt.transpose(2, 1, 0).reshape(2, 128, TBL)
        ).astype(np.float16)
        in_maps.append(
            {
                "xs": xs,
                "wih": wih_t,
                "whh": whh_t,
                "biasg": biasg,
                "b2row": b2row,
                "ones": ones,
                "id32": id32,
                "id16": id16,
            }
        )
    res = run_bass_kernel_spmd(nc, in_maps, core_ids=list(range(NCORES)))
    if getattr(res, "exec_time_ns", None):
        print("HW exec time:", res.exec_time_ns, "ns")
    global LAST_RESULT
    LAST_RESULT = res
    out = np.concatenate([res.results[c]["out"] for c in range(NCORES)], axis=0)
    return out.astype(np.float32)


LAST_RESULT = None


if __name__ == "__main__":
    rng = np.random.default_rng(0)
    ins = {
        "x": rng.standard_normal((B, T, X)).astype(np.float32),
        "W_ih": rng.standard_normal((G3, X)).astype(np.float32) / 32,
        "W_hh": rng.standard_normal((G3, H)).astype(np.float32) / 32,
        "b_ih": rng.standard_normal(G3).astype(np.float32) / 32,
        "b_hh": rng.standard_normal(G3).astype(np.float32) / 32,
        "w1": rng.integers(0, 2, T).astype(np.int32),
        "w2": rng.integers(0, 2, T).astype(np.int32),
        "skip_size": 5,
    }
    ins["w2"] = np.where(ins["w1"] == 0, 1, ins["w2"]).astype(np.int32)
    out = kernel(**ins)
    print("ran", out.shape, out.dtype, float(np.abs(out).mean()))
